# revision 36
# baseline (speedup 1.0000x reference)
"""Chamfer distance kernel for 8 Trainium2 NeuronCores.

Problem: x, y: [4, 8192, 3] f32 point clouds.
  D[b,i,j] = ||x[b,i] - y[b,j]||^2
  out = mean_{b,i} min_j sqrt(D) + mean_{b,j} min_i sqrt(D)

Strategy (v4, banded): both clouds are z-sorted on the host.  An x-chunk of
128 consecutive-z points only computes distances against a window of W=8
y-slabs (1024 points) around its own z rank, shrinking the distance matrix
8x (8192x1024 band per batch-half instead of 8192x4096).  Exactness is
restored on the host: any point whose banded min exceeds its window's
z-margin lower bound (D >= dz^2 for out-of-window points) is recomputed
exactly in numpy (~0.35% of points).

Device pipeline per chunk (negated domain, PE emits -D, reductions are MAX):
  - PE: one K=13 fp16 compensated matmul pair -> PSUM [128, 1024] f32.
  - ACT: Copy drain PSUM -> SBUF fp16 (1 el/cyc, the only cheap PSUM exit).
  - DVE: tensor_tensor_reduce(max, max) gives the full row max in one op;
    a shifted tensor_tensor(max) accumulates the column max in true-j space.

Core (b, h) takes batch b, x-half h.  h=1 cores receive z-DESCENDING data so
the window offsets (max(c-3,0)*128) are identical across cores -> one SPMD
program.  Host mirrors h=1 results back.
"""

import sys

if "/opt/trn_rl_repo" not in sys.path:
    sys.path.insert(0, "/opt/trn_rl_repo")

import numpy as np


def _install_ntff_hook_shim():
    """The agent image's antenv lacks axon_hooks; bass_utils imports it when
    BASS_TRACE is set. Register a stand-in backed by the ctypes NTFF hook."""
    import types

    if "antenv.axon_hooks" in sys.modules:
        return
    try:
        import antenv
        from trn_agent_boot.trn_boot import _ntff_profile_via_ctypes
    except ImportError:
        return
    mod = types.ModuleType("antenv.axon_hooks")
    _hook = [None]

    def set_axon_ntff_profile_hook(h):
        _hook[0] = h

    def get_axon_ntff_profile_hook():
        if _hook[0] is None:
            try:
                _hook[0] = _ntff_profile_via_ctypes("/opt/axon/libaxon_pjrt.so")
            except Exception:
                return None
        return _hook[0]

    mod.set_axon_ntff_profile_hook = set_axon_ntff_profile_hook
    mod.get_axon_ntff_profile_hook = get_axon_ntff_profile_hook
    sys.modules["antenv.axon_hooks"] = mod
    antenv.axon_hooks = mod


_install_ntff_hook_shim()

import concourse.bacc as bacc
import concourse.bass as bass
import concourse.mybir as mybir
import concourse.tile as tile
from concourse.bass_utils import run_bass_kernel_spmd

BS = 4
N = 8192
SL = 128               # slab size
NS = N // SL           # 64 slabs
W = 3                  # window width in slabs
BACK = 1               # window starts BACK slabs below the chunk's own slab
FD = W * SL            # 768 columns per chunk
NCH = 32               # x-chunks per core (half of 64)
COLW = (NCH - 1 - BACK) * SL + FD   # colacc width per core
N_CORES = 8
K = 13                 # augmented contraction dim

F32 = mybir.dt.float32
F16 = mybir.dt.float16
MAX_OP = mybir.AluOpType.max
COPY_FN = mybir.ActivationFunctionType.Copy

NEG_INIT = -60000.0    # below any real -D (D <= ~60 for this data)

USE_TTR = False        # tensor_tensor_reduce(max) row path (hung HW once)
RT = FD // 2           # row-tail width after 1 TT-tree level

LAST_RESULTS = None
_compiled_nc = None


def _s0(c):
    return max(c - BACK, 0) * SL


def _build_program():
    nc = bacc.Bacc()

    xa = nc.declare_dram_parameter("xa", [K, NCH * SL], F16, isOutput=False)
    ya = nc.declare_dram_parameter("ya", [K, COLW], F16, isOutput=False)
    d16_out = nc.declare_dram_parameter("d16", [128, NCH, FD], F16, isOutput=True)
    colacc_out = nc.declare_dram_parameter("colacc", [128, COLW], F16, isOutput=True)

    with tile.TileContext(nc) as tc:
        with (
            tc.tile_pool(name="const", bufs=1) as const_pool,
            tc.tile_pool(name="acc", bufs=1) as acc_pool,
            tc.tile_pool(name="d16", bufs=3) as d16_pool,
            tc.tile_pool(name="psum", bufs=4, space="PSUM") as psum_pool,
        ):
            xa_sb = const_pool.tile([K, NCH * SL], F16, tag="xa")
            ya_sb = const_pool.tile([K, COLW], F16, tag="ya")
            # prefetch order, spread across engine DMA queues so the
            # descriptor generations run in parallel; tiny first pieces
            # unblock the first matmul as early as possible
            nc.scalar.dma_start(xa_sb[:, 0:256], xa[:, 0:256])
            nc.sync.dma_start(ya_sb[:, 0:2 * FD], ya[:, 0:2 * FD])
            nc.gpsimd.dma_start(xa_sb[:, 256:], xa[:, 256:])
            nc.sync.dma_start(ya_sb[:, 2 * FD:], ya[:, 2 * FD:])

            # HAM warm-up operand: the PE chews on this during the input-DMA
            # dead window so the real matmuls run at 2.4 GHz, not 1.2 GHz
            dummy = const_pool.tile([K, 512], F16, tag="warm")
            nc.vector.memset(dummy[:], 1.0)

            colacc = acc_pool.tile([128, COLW], F16, tag="colacc")
            # split so the first piece (which gates the first col TT)
            # finishes before the pipeline reaches it
            nc.vector.memset(colacc[:, 0:COLW // 2], NEG_INIT)
            nc.vector.memset(colacc[:, COLW // 2:], NEG_INIT)

            colacc_sent = 0
            for t in range(NCH // 2):
                # psum padded to one full bank per chunk (matmul output may
                # not cross a bank boundary)
                ps = psum_pool.tile([128, 2, 512], F32)
                if t == 0:
                    # warm-up burst; overwritten by the real matmul below
                    for _ in range(6):
                        nc.tensor.matmul(
                            ps[:, 0, :], dummy[:, 0:128], dummy[:, :],
                            start=True, stop=True,
                        )
                for u in range(2):
                    c = 2 * t + u
                    nc.tensor.matmul(
                        ps[:, u, 0:FD],
                        xa_sb[:, c * 128:(c + 1) * 128],
                        ya_sb[:, _s0(c): _s0(c) + FD],
                        start=True, stop=True,
                    )
                d16 = d16_pool.tile([128, 2, FD], F16)
                nc.scalar.activation(d16[:], ps[:, :, 0:FD], COPY_FN)
                # raw banded tiles go to the host, which computes the row
                # mins from them (no on-device row tree)
                if t % 2 == 0:
                    nc.sync.dma_start(d16_out[:, 2 * t:2 * t + 2, :], d16[:])
                else:
                    nc.gpsimd.dma_start(d16_out[:, 2 * t:2 * t + 2, :], d16[:])
                # shifted column-max accumulate in true-j space
                for u in range(2):
                    c = 2 * t + u
                    off = _s0(c)
                    nc.vector.tensor_tensor(
                        colacc[:, off:off + FD], colacc[:, off:off + FD],
                        d16[:, u, :], MAX_OP,
                    )
                # stream out finalized colacc prefix (later windows never
                # touch [0, off(c)) again)
                c = 2 * t + 1
                if c in (13, 21, 29):
                    off = _s0(c)
                    nc.sync.dma_start(
                        colacc_out[:, colacc_sent:off],
                        colacc[:, colacc_sent:off],
                    )
                    colacc_sent = off

            nc.sync.dma_start(colacc_out[:, colacc_sent:], colacc[:, colacc_sent:])

    nc.compile()
    return nc


def _augment(x, y):
    """Compensated fp16 augmentation for -D, K=13.

    xaugT: [13, n] rows (u_h, u_l, b_h(3), b_l(3), b_h(3), -1, -1)
      with u = -xx, b = 2x
    yaugT: [13, m] rows (1, 1, y_h(3), y_h(3), y_l(3), yy_h, yy_l)
    so sum_k xa[k]ya[k] = -xx - yy + 2x.y = -D up to the dropped
    b_l*y_l term (~1e-6).  x: [n, 3], y: [m, 3] float64.
    """
    u = -(x * x).sum(-1)
    yy = (y * y).sum(-1)
    b = 2.0 * x
    bh = b.astype(np.float16)
    bl = (b - bh.astype(np.float64)).astype(np.float16)
    yh = y.astype(np.float16)
    yl = (y - yh.astype(np.float64)).astype(np.float16)
    uh = u.astype(np.float16)
    ul = (u - uh.astype(np.float64)).astype(np.float16)
    yyh = yy.astype(np.float16)
    yyl = (yy - yyh.astype(np.float64)).astype(np.float16)
    ones_x = np.ones(x.shape[0], dtype=np.float16)
    ones_y = np.ones(y.shape[0], dtype=np.float16)

    xaug = np.stack([
        uh, ul,
        bh[:, 0], bh[:, 1], bh[:, 2],
        bl[:, 0], bl[:, 1], bl[:, 2],
        bh[:, 0], bh[:, 1], bh[:, 2],
        -ones_x, -ones_x,
    ], axis=0)
    yaug = np.stack([
        ones_y, ones_y,
        yh[:, 0], yh[:, 1], yh[:, 2],
        yh[:, 0], yh[:, 1], yh[:, 2],
        yl[:, 0], yl[:, 1], yl[:, 2],
        yyh, yyl,
    ], axis=0)
    return xaug, yaug


def _windows_asc():
    """Per global ascending x-chunk g: covered y-slab interval [lo, hi)."""
    wins = []
    for g in range(NS):
        if g < NCH:
            s0 = max(g - BACK, 0)
            wins.append((s0, s0 + W))
        else:
            c = NS - 1 - g
            s0 = max(c - BACK, 0)
            wins.append((NS - s0 - W, NS - s0))
    return wins


def kernel(x, y):
    global LAST_RESULTS, _compiled_nc

    x = np.asarray(x, dtype=np.float32)
    y = np.asarray(y, dtype=np.float32)
    bs, n, d = x.shape
    assert (bs, n, d) == (BS, N, 3), (bs, n, d)

    x64 = x.astype(np.float64)
    y64 = y.astype(np.float64)

    # z-ascending permutations per batch
    px = [np.argsort(x64[b, :, 2], kind="stable") for b in range(BS)]
    py = [np.argsort(y64[b, :, 2], kind="stable") for b in range(BS)]
    xs = [x64[b][px[b]] for b in range(BS)]   # ascending-z sorted clouds
    ys = [y64[b][py[b]] for b in range(BS)]

    in_maps = []
    for core in range(N_CORES):
        b, h = divmod(core, 2)
        if h == 0:
            xc, yc = xs[b], ys[b]
        else:
            xc, yc = xs[b][::-1], ys[b][::-1]
        xaug, yaug = _augment(xc[: NCH * SL], yc[:COLW])
        in_maps.append({
            "xa": np.ascontiguousarray(xaug),
            "ya": np.ascontiguousarray(yaug),
        })

    if _compiled_nc is None:
        _compiled_nc = _build_program()

    res = None
    last_err = None
    for attempt in range(4):
        try:
            res = run_bass_kernel_spmd(_compiled_nc, in_maps, list(range(N_CORES)))
            break
        except Exception as e:  # transient axon/NRT hiccups: rebuild + retry
            last_err = e
            if "axon_start_nrt_profile" in repr(e):
                import os
                os.environ["BASS_NEVER_TRACE"] = "1"
            else:
                _compiled_nc = _build_program()
    if res is None:
        raise last_err
    LAST_RESULTS = res

    wins = _windows_asc()
    # coverage interval of x-chunks per y-slab
    gmin = np.full(NS, NS, dtype=np.int64)
    gmax = np.full(NS, -1, dtype=np.int64)
    for g, (lo, hi) in enumerate(wins):
        for s in range(lo, hi):
            gmin[s] = min(gmin[s], g)
            gmax[s] = max(gmax[s], g)

    vals1 = np.empty((BS, N), dtype=np.float64)
    vals2 = np.empty((BS, N), dtype=np.float64)
    for b in range(BS):
        v1 = np.empty(N, dtype=np.float64)
        v2 = np.full(N, -np.inf, dtype=np.float64)
        for h in range(2):
            r = res.results[2 * b + h]
            rm = r["d16"].astype(np.float32).max(axis=2)     # [128, 32]
            seg1 = rm.T.reshape(-1).astype(np.float64)       # core-x 128c+p
            cm = r["colacc"].astype(np.float64).max(axis=0)  # [COLW]
            if h == 0:
                v1[:NCH * SL] = seg1
                v2[:COLW] = np.maximum(v2[:COLW], cm)
            else:
                v1[NCH * SL:] = seg1[::-1]
                v2[N - COLW:] = np.maximum(v2[N - COLW:], cm[::-1])
        v1_sq = np.maximum(-v1, 0.0)
        v2_sq = np.maximum(-v2, 0.0)

        xz, yz = xs[b][:, 2], ys[b][:, 2]
        # row margins: out-of-window y's satisfy D >= (z_x - boundary)^2
        m1 = np.full(N, np.inf)
        for g, (lo, hi) in enumerate(wins):
            zi = xz[g * SL:(g + 1) * SL]
            mg = np.full(SL, np.inf)
            if lo > 0:
                mg = np.minimum(mg, np.maximum(zi - yz[lo * SL - 1], 0.0) ** 2)
            if hi < NS:
                mg = np.minimum(mg, np.maximum(yz[hi * SL] - zi, 0.0) ** 2)
            m1[g * SL:(g + 1) * SL] = mg
        # col margins
        m2 = np.full(N, np.inf)
        for s in range(NS):
            zj = yz[s * SL:(s + 1) * SL]
            mg = np.full(SL, np.inf)
            if gmin[s] > 0:
                mg = np.minimum(mg, np.maximum(zj - xz[gmin[s] * SL - 1], 0.0) ** 2)
            if gmax[s] < NS - 1:
                mg = np.minimum(mg, np.maximum(xz[(gmax[s] + 1) * SL] - zj, 0.0) ** 2)
            m2[s * SL:(s + 1) * SL] = mg

        viol1 = np.nonzero(v1_sq * 1.001 + 1e-5 >= m1)[0]
        viol2 = np.nonzero(v2_sq * 1.001 + 1e-5 >= m2)[0]
        for i0 in range(0, len(viol1), 512):
            idx = viol1[i0:i0 + 512]
            dd = ((xs[b][idx, None, :] - ys[b][None, :, :]) ** 2).sum(-1)
            v1_sq[idx] = dd.min(1)
        for i0 in range(0, len(viol2), 512):
            idx = viol2[i0:i0 + 512]
            dd = ((ys[b][idx, None, :] - xs[b][None, :, :]) ** 2).sum(-1)
            v2_sq[idx] = dd.min(1)

        vals1[b] = np.sqrt(v1_sq)
        vals2[b] = np.sqrt(v2_sq)

    out = vals1.mean(axis=1).mean() + vals2.mean(axis=1).mean()
    return np.float32(out)


# revision 38
# speedup vs baseline: 1.1379x; 1.1379x over previous
"""Chamfer distance kernel for 8 Trainium2 NeuronCores.

Problem: x, y: [4, 8192, 3] f32 point clouds.
  D[b,i,j] = ||x[b,i] - y[b,j]||^2
  out = mean_{b,i} min_j sqrt(D) + mean_{b,j} min_i sqrt(D)

Strategy (v6, banded): both clouds are z-sorted on the host.  An x-chunk of
128 consecutive-z points only computes distances against a window of W=3
y-slabs (384 points) starting one slab below its own rank, shrinking the
distance matrix ~21x (8192x384 band per batch-half instead of 8192x8192).
Exactness is restored on the host: any point whose banded min exceeds its
window's z-margin lower bound (D >= dz^2 for any out-of-window point) is
recomputed exactly in numpy (~1.4% of points).

Device pipeline per chunk pair (negated domain: PE emits -D, reductions MAX):
  - PE: K=13 fp16 compensated matmuls -> PSUM [128, 2, 384(pad 512)] f32.
  - ACT: one Copy drain per pair, PSUM -> SBUF fp16 (the only cheap PSUM
    exit; 1 elem/cycle at 1.2 GHz).
  - DVE: one merged row-max tree level per pair (width-192 tails finish on
    host) + per-chunk shifted tensor_tensor(max) column accumulation in
    true-j space; colacc prefixes stream out as their windows finalize.

Core (b, h) takes batch b, x-half h.  h=1 cores receive z-DESCENDING data so
the window offsets (max(c-1,0)*128) are identical across cores -> one SPMD
program.  Host mirrors h=1 results back, merges the two halves' column
maxes, applies the margin-based exact fixup, and takes sqrt/means.
"""

import sys

if "/opt/trn_rl_repo" not in sys.path:
    sys.path.insert(0, "/opt/trn_rl_repo")

import numpy as np


def _install_ntff_hook_shim():
    """The agent image's antenv lacks axon_hooks; bass_utils imports it when
    BASS_TRACE is set. Register a stand-in backed by the ctypes NTFF hook."""
    import types

    if "antenv.axon_hooks" in sys.modules:
        return
    try:
        import antenv
        from trn_agent_boot.trn_boot import _ntff_profile_via_ctypes
    except ImportError:
        return
    mod = types.ModuleType("antenv.axon_hooks")
    _hook = [None]

    def set_axon_ntff_profile_hook(h):
        _hook[0] = h

    def get_axon_ntff_profile_hook():
        if _hook[0] is None:
            try:
                _hook[0] = _ntff_profile_via_ctypes("/opt/axon/libaxon_pjrt.so")
            except Exception:
                return None
        return _hook[0]

    mod.set_axon_ntff_profile_hook = set_axon_ntff_profile_hook
    mod.get_axon_ntff_profile_hook = get_axon_ntff_profile_hook
    sys.modules["antenv.axon_hooks"] = mod
    antenv.axon_hooks = mod


_install_ntff_hook_shim()

import concourse.bacc as bacc
import concourse.bass as bass
import concourse.mybir as mybir
import concourse.tile as tile
from concourse.bass_utils import run_bass_kernel_spmd

BS = 4
N = 8192
SL = 128               # slab size
NS = N // SL           # 64 slabs
W = 3                  # window width in slabs
BACK = 1               # window starts BACK slabs below the chunk's own slab
FD = W * SL            # 768 columns per chunk
NCH = 32               # x-chunks per core (half of 64)
COLW = (NCH - 1 - BACK) * SL + FD   # colacc width per core
N_CORES = 8
K = 13                 # augmented contraction dim

F32 = mybir.dt.float32
F16 = mybir.dt.float16
MAX_OP = mybir.AluOpType.max
COPY_FN = mybir.ActivationFunctionType.Copy

NEG_INIT = -60000.0    # below any real -D (D <= ~60 for this data)

USE_TTR = False        # tensor_tensor_reduce(max) row path (hung HW once)
RT = FD // 2           # row-tail width after 1 TT-tree level

LAST_RESULTS = None
_compiled_nc = None


def _s0(c):
    return max(c - BACK, 0) * SL


def _build_program():
    nc = bacc.Bacc()

    xa = nc.declare_dram_parameter("xa", [K, NCH * SL], F16, isOutput=False)
    ya = nc.declare_dram_parameter("ya", [K, COLW], F16, isOutput=False)
    if USE_TTR:
        rowmax_out = nc.declare_dram_parameter("rowmax", [128, NCH], F32, isOutput=True)
    else:
        rowmax_out = nc.declare_dram_parameter("rowmax", [128, NCH, RT], F16, isOutput=True)
    colacc_out = nc.declare_dram_parameter("colacc", [128, COLW], F16, isOutput=True)

    with tile.TileContext(nc) as tc:
        with (
            tc.tile_pool(name="const", bufs=1) as const_pool,
            tc.tile_pool(name="acc", bufs=1) as acc_pool,
            tc.tile_pool(name="d16", bufs=3) as d16_pool,
            tc.tile_pool(name="scr", bufs=3) as scr_pool,
            tc.tile_pool(name="psum", bufs=4, space="PSUM") as psum_pool,
        ):
            xa_sb = const_pool.tile([K, NCH * SL], F16, tag="xa")
            ya_sb = const_pool.tile([K, COLW], F16, tag="ya")
            # prefetch order, spread across engine DMA queues so the
            # descriptor generations run in parallel; tiny first pieces
            # unblock the first matmul as early as possible
            nc.scalar.dma_start(xa_sb[:, 0:256], xa[:, 0:256])
            nc.sync.dma_start(ya_sb[:, 0:2 * FD], ya[:, 0:2 * FD])
            nc.gpsimd.dma_start(xa_sb[:, 256:], xa[:, 256:])
            nc.sync.dma_start(ya_sb[:, 2 * FD:], ya[:, 2 * FD:])

            colacc = acc_pool.tile([128, COLW], F16, tag="colacc")
            # split so the first piece (which gates the first col TT)
            # finishes before the pipeline reaches it
            nc.vector.memset(colacc[:, 0:COLW // 2], NEG_INIT)
            nc.vector.memset(colacc[:, COLW // 2:], NEG_INIT)
            if USE_TTR:
                rowacc = acc_pool.tile([128, NCH], F32, tag="rowacc")

            colacc_sent = 0
            for t in range(NCH // 2):
                # psum padded to one full bank per chunk (matmul output may
                # not cross a bank boundary)
                ps = psum_pool.tile([128, 2, 512], F32)
                for u in range(2):
                    c = 2 * t + u
                    nc.tensor.matmul(
                        ps[:, u, 0:FD],
                        xa_sb[:, c * 128:(c + 1) * 128],
                        ya_sb[:, _s0(c): _s0(c) + FD],
                        start=True, stop=True,
                    )
                d16 = d16_pool.tile([128, 2, FD], F16)
                nc.scalar.activation(d16[:], ps[:, :, 0:FD], COPY_FN)
                # merged 1-level row max tree for both chunks; width-RT
                # tails finish on host
                scr = scr_pool.tile([128, 2, RT], F16)
                nc.vector.tensor_tensor(
                    scr[:], d16[:, :, 0:RT], d16[:, :, RT:2 * RT], MAX_OP
                )
                nc.sync.dma_start(rowmax_out[:, 2 * t:2 * t + 2, :], scr[:])
                # shifted column-max accumulate in true-j space
                for u in range(2):
                    c = 2 * t + u
                    off = _s0(c)
                    nc.vector.tensor_tensor(
                        colacc[:, off:off + FD], colacc[:, off:off + FD],
                        d16[:, u, :], MAX_OP,
                    )
                # stream out finalized colacc prefix (later windows never
                # touch [0, off(c)) again)
                c = 2 * t + 1
                if c in (13, 21, 29):
                    off = _s0(c)
                    nc.sync.dma_start(
                        colacc_out[:, colacc_sent:off],
                        colacc[:, colacc_sent:off],
                    )
                    colacc_sent = off

            nc.sync.dma_start(colacc_out[:, colacc_sent:], colacc[:, colacc_sent:])

    nc.compile()
    return nc


def _augment(x, y):
    """Compensated fp16 augmentation for -D, K=13.

    xaugT: [13, n] rows (u_h, u_l, b_h(3), b_l(3), b_h(3), -1, -1)
      with u = -xx, b = 2x
    yaugT: [13, m] rows (1, 1, y_h(3), y_h(3), y_l(3), yy_h, yy_l)
    so sum_k xa[k]ya[k] = -xx - yy + 2x.y = -D up to the dropped
    b_l*y_l term (~1e-6).  x: [n, 3], y: [m, 3] float64.
    """
    u = -(x * x).sum(-1)
    yy = (y * y).sum(-1)
    b = 2.0 * x
    bh = b.astype(np.float16)
    bl = (b - bh.astype(np.float64)).astype(np.float16)
    yh = y.astype(np.float16)
    yl = (y - yh.astype(np.float64)).astype(np.float16)
    uh = u.astype(np.float16)
    ul = (u - uh.astype(np.float64)).astype(np.float16)
    yyh = yy.astype(np.float16)
    yyl = (yy - yyh.astype(np.float64)).astype(np.float16)
    ones_x = np.ones(x.shape[0], dtype=np.float16)
    ones_y = np.ones(y.shape[0], dtype=np.float16)

    xaug = np.stack([
        uh, ul,
        bh[:, 0], bh[:, 1], bh[:, 2],
        bl[:, 0], bl[:, 1], bl[:, 2],
        bh[:, 0], bh[:, 1], bh[:, 2],
        -ones_x, -ones_x,
    ], axis=0)
    yaug = np.stack([
        ones_y, ones_y,
        yh[:, 0], yh[:, 1], yh[:, 2],
        yh[:, 0], yh[:, 1], yh[:, 2],
        yl[:, 0], yl[:, 1], yl[:, 2],
        yyh, yyl,
    ], axis=0)
    return xaug, yaug


def _windows_asc():
    """Per global ascending x-chunk g: covered y-slab interval [lo, hi)."""
    wins = []
    for g in range(NS):
        if g < NCH:
            s0 = max(g - BACK, 0)
            wins.append((s0, s0 + W))
        else:
            c = NS - 1 - g
            s0 = max(c - BACK, 0)
            wins.append((NS - s0 - W, NS - s0))
    return wins


def kernel(x, y):
    global LAST_RESULTS, _compiled_nc

    x = np.asarray(x, dtype=np.float32)
    y = np.asarray(y, dtype=np.float32)
    bs, n, d = x.shape
    assert (bs, n, d) == (BS, N, 3), (bs, n, d)

    x64 = x.astype(np.float64)
    y64 = y.astype(np.float64)

    # z-ascending permutations per batch
    px = [np.argsort(x64[b, :, 2], kind="stable") for b in range(BS)]
    py = [np.argsort(y64[b, :, 2], kind="stable") for b in range(BS)]
    xs = [x64[b][px[b]] for b in range(BS)]   # ascending-z sorted clouds
    ys = [y64[b][py[b]] for b in range(BS)]

    in_maps = []
    for core in range(N_CORES):
        b, h = divmod(core, 2)
        if h == 0:
            xc, yc = xs[b], ys[b]
        else:
            xc, yc = xs[b][::-1], ys[b][::-1]
        xaug, yaug = _augment(xc[: NCH * SL], yc[:COLW])
        in_maps.append({
            "xa": np.ascontiguousarray(xaug),
            "ya": np.ascontiguousarray(yaug),
        })

    if _compiled_nc is None:
        _compiled_nc = _build_program()

    res = None
    last_err = None
    for attempt in range(4):
        try:
            res = run_bass_kernel_spmd(_compiled_nc, in_maps, list(range(N_CORES)))
            break
        except Exception as e:  # transient axon/NRT hiccups: rebuild + retry
            last_err = e
            if "axon_start_nrt_profile" in repr(e):
                import os
                os.environ["BASS_NEVER_TRACE"] = "1"
            else:
                _compiled_nc = _build_program()
    if res is None:
        raise last_err
    LAST_RESULTS = res

    wins = _windows_asc()
    # coverage interval of x-chunks per y-slab
    gmin = np.full(NS, NS, dtype=np.int64)
    gmax = np.full(NS, -1, dtype=np.int64)
    for g, (lo, hi) in enumerate(wins):
        for s in range(lo, hi):
            gmin[s] = min(gmin[s], g)
            gmax[s] = max(gmax[s], g)

    vals1 = np.empty((BS, N), dtype=np.float64)
    vals2 = np.empty((BS, N), dtype=np.float64)
    for b in range(BS):
        v1 = np.empty(N, dtype=np.float64)
        v2 = np.full(N, -np.inf, dtype=np.float64)
        for h in range(2):
            r = res.results[2 * b + h]
            rm = r["rowmax"].astype(np.float64)
            if not USE_TTR:                              # [128, 32, RT] tails
                rm = rm.max(axis=2)
            seg1 = rm.T.reshape(-1)                      # core-x order 128c+p
            cm = r["colacc"].astype(np.float64).max(axis=0)  # [COLW]
            if h == 0:
                v1[:NCH * SL] = seg1
                v2[:COLW] = np.maximum(v2[:COLW], cm)
            else:
                v1[NCH * SL:] = seg1[::-1]
                v2[N - COLW:] = np.maximum(v2[N - COLW:], cm[::-1])
        v1_sq = np.maximum(-v1, 0.0)
        v2_sq = np.maximum(-v2, 0.0)

        xz, yz = xs[b][:, 2], ys[b][:, 2]
        # row margins: out-of-window y's satisfy D >= (z_x - boundary)^2
        m1 = np.full(N, np.inf)
        for g, (lo, hi) in enumerate(wins):
            zi = xz[g * SL:(g + 1) * SL]
            mg = np.full(SL, np.inf)
            if lo > 0:
                mg = np.minimum(mg, np.maximum(zi - yz[lo * SL - 1], 0.0) ** 2)
            if hi < NS:
                mg = np.minimum(mg, np.maximum(yz[hi * SL] - zi, 0.0) ** 2)
            m1[g * SL:(g + 1) * SL] = mg
        # col margins
        m2 = np.full(N, np.inf)
        for s in range(NS):
            zj = yz[s * SL:(s + 1) * SL]
            mg = np.full(SL, np.inf)
            if gmin[s] > 0:
                mg = np.minimum(mg, np.maximum(zj - xz[gmin[s] * SL - 1], 0.0) ** 2)
            if gmax[s] < NS - 1:
                mg = np.minimum(mg, np.maximum(xz[(gmax[s] + 1) * SL] - zj, 0.0) ** 2)
            m2[s * SL:(s + 1) * SL] = mg

        viol1 = np.nonzero(v1_sq * 1.001 + 1e-5 >= m1)[0]
        viol2 = np.nonzero(v2_sq * 1.001 + 1e-5 >= m2)[0]
        for i0 in range(0, len(viol1), 512):
            idx = viol1[i0:i0 + 512]
            dd = ((xs[b][idx, None, :] - ys[b][None, :, :]) ** 2).sum(-1)
            v1_sq[idx] = dd.min(1)
        for i0 in range(0, len(viol2), 512):
            idx = viol2[i0:i0 + 512]
            dd = ((ys[b][idx, None, :] - xs[b][None, :, :]) ** 2).sum(-1)
            v2_sq[idx] = dd.min(1)

        vals1[b] = np.sqrt(v1_sq)
        vals2[b] = np.sqrt(v2_sq)

    out = vals1.mean(axis=1).mean() + vals2.mean(axis=1).mean()
    return np.float32(out)
